# revision 13
# baseline (speedup 1.0000x reference)
# ChildSumTreeLSTM on a complete binary tree (heap order), Trainium2 Bass kernel.
#
# Strategy: the heap-ordered tree decomposes into 64 independent subtrees
# rooted at global depth 6; core k owns subtrees 8k..8k+7 as one contiguous
# chunk per level (children of a core's nodes stay inside the core's chunk,
# child pairs stay adjacent).  Zero cross-core communication.  The 63-node
# tree top plus the 64 subtree-root forget gates run on the host in fp32.
#
# Per-core device pipeline ("transposed" layout: hidden dim on SBUF
# partitions, nodes on the free dim), bottom-up over 11 local levels
# (8192..8 nodes):
#   z_iou = Wx_aug @ [x;1] + Wh @ h_sum    (PSUM, bf16; biases folded into
#                                           an extra ones-row of x)
#   i,o   = SIG(z)                          one fused act over 4 chunks
#   u     = 2*SIG(2 z)-1                    (tanh via sigmoid: one table set
#                                           for the whole kernel)
#   c = i*u + fc_sum ; h = o*(2*SIG(2c)-1)
#   f = SIG(Wfx_aug @ [x_par;1] [col-doubled] + Wfh @ h)
#   h_sum/fc_sum pairwise child adds (strided views)
#   logits = Wout^T-stationary matmul -> [5, n] PSUM -> staged [6, nloc]
# Epilogue (once): EXP over all logits, ones-matmul partition sum,
# LN -> per-node logsumexp in row 5.  Host: out = logits - lse, top levels.
import numpy as np
import ml_dtypes

E, H, L, DEPTH = 300, 256, 5, 17
NCORES = 8
CORE_DEPTH = 8                  # 8 local levels per core: 8192 .. 64
TILE = 256

BF16 = ml_dtypes.bfloat16


def _level_sizes(core_depth):
    return [1 << (13 - i) for i in range(core_depth)]  # leaf 8192 first


def _level_offsets(sizes):
    offs, o = [], 0
    for n in sizes:
        offs.append(o)
        o += n
    return offs, o


# ---------------------------------------------------------------------------
# Device kernel builder
# ---------------------------------------------------------------------------
_NC_CACHE = {}


def build_nc(core_depth=CORE_DEPTH, repeats=1, x_resident=False):
    """Build + compile the per-core Bass program (SPMD across 8 cores)."""
    key = (core_depth, repeats, x_resident)
    if key in _NC_CACHE:
        return _NC_CACHE[key]
    import concourse.bacc as bacc
    import concourse.mybir as mybir
    import concourse.tile as tile

    fp32 = mybir.dt.float32
    bf16 = mybir.dt.bfloat16
    SIG = mybir.ActivationFunctionType.Sigmoid
    EXP = mybir.ActivationFunctionType.Exp
    LN = mybir.ActivationFunctionType.Ln
    MUL = mybir.AluOpType.mult
    ADD = mybir.AluOpType.add

    sizes = _level_sizes(core_depth)
    offs, nloc = _level_offsets(sizes)
    nroot = sizes[-1]            # 8 subtree roots per core

    nc = bacc.Bacc("TRN2", target_bir_lowering=False, debug=False,
                   num_devices=NCORES)
    xk = nc.dram_tensor("xk", [128, 3 * nloc], bf16, kind="ExternalInput")
    wx = nc.dram_tensor("wx", [128, 3 * 1024], bf16, kind="ExternalInput")
    wh = nc.dram_tensor("wh", [128, 2 * 1024], bf16, kind="ExternalInput")
    wo = nc.dram_tensor("wo", [128, 10], bf16, kind="ExternalInput")
    bout5 = nc.dram_tensor("bout5", [5, 1], fp32, kind="ExternalInput")
    out5 = nc.dram_tensor("out5", [6, nloc], bf16, kind="ExternalOutput")
    outhc = nc.dram_tensor("outhc", [128, 4 * nroot], fp32,
                           kind="ExternalOutput")

    xk_v = xk.ap().rearrange("p (k n) -> p k n", k=3)
    wx_v = wx.ap().rearrange("p (k m) -> p k m", k=3)
    wh_v = wh.ap().rearrange("p (k m) -> p k m", k=2)

    EPSC = 2048                  # epilogue psum chunk (4 banks)

    with tile.TileContext(nc) as tc:
        with tc.tile_pool(name="wpool", bufs=1) as wpool, \
             tc.tile_pool(name="xpool", bufs=3) as xpool, \
             tc.tile_pool(name="gpool", bufs=2) as gpool, \
             tc.tile_pool(name="hpool", bufs=2) as hpool, \
             tc.tile_pool(name="spool", bufs=2) as spool, \
             tc.tile_pool(name="stpool", bufs=1) as stpool:

            # --- load weights once ---
            wx_sb = wpool.tile([128, 3, 1024], bf16, tag="wx")
            wh_sb = wpool.tile([128, 2, 1024], bf16, tag="wh")
            wo_sb = wpool.tile([128, 10], bf16, tag="wo")
            bout5_sb = wpool.tile([5, 1], fp32, tag="bout5")
            ones5_sb = wpool.tile([5, 1], bf16, tag="ones5")
            nc.sync.dma_start(wx_sb[:], wx_v[:])
            nc.sync.dma_start(wh_sb[:], wh_v[:])
            nc.sync.dma_start(wo_sb[:], wo.ap())
            nc.sync.dma_start(bout5_sb[:], bout5.ap())
            nc.vector.memset(ones5_sb[:], 1.0)
            if x_resident:
                xt_g = wpool.tile([128, 3, TILE], bf16, tag="xt_g")
                xp_g = wpool.tile([128, 3, TILE // 2], bf16, tag="xp_g")
                nc.sync.dma_start(xt_g[:], xk_v[:, :, 0:TILE])
                nc.sync.dma_start(xp_g[:], xk_v[:, :, 0:TILE // 2])

            def body():
                stage_sb = stpool.tile([33, nloc], bf16, tag="stage")
                lse_sb = stage_sb[32:33, :]
                outhc_sb = stpool.tile([128, 4 * nroot], fp32, tag="outhc")
                with tc.tile_pool(name="piou", bufs=2, space="PSUM") as piou, \
                     tc.tile_pool(name="pfl", bufs=2, space="PSUM") as pfl:
                    hsum_cur = fcsum_cur = None
                    for lvl, n in enumerate(sizes):
                        off = offs[lvl]
                        is_leaf = lvl == 0
                        is_root = lvl == core_depth - 1
                        n2 = n // 2
                        if not is_root:
                            hsum_next = spool.tile([128, 2, max(n2, 1)],
                                                   bf16, tag="hsum")
                            fcsum_next = spool.tile([128, 2, max(n2, 1)],
                                                    bf16, tag="fcsum")
                        ntiles = (n + TILE - 1) // TILE
                        for t in range(ntiles):
                            t0 = t * TILE
                            tn = min(TILE, n - t0)
                            pn = max(tn // 2, 1)
                            # -- x tile loads (bf16, 3 K-chunks) --
                            if x_resident:
                                xt, xp = xt_g, xp_g
                            else:
                                xt = xpool.tile([128, 3, TILE], bf16,
                                                tag="xt")
                                nc.sync.dma_start(
                                    xt[:, :, :tn],
                                    xk_v[:, :, off + t0: off + t0 + tn])
                                if not is_root:
                                    xp = xpool.tile([128, 3, TILE // 2],
                                                    bf16, tag="xp")
                                    p0 = offs[lvl + 1] + t0 // 2
                                    nc.sync.dma_start(
                                        xp[:, :, :pn],
                                        xk_v[:, :, p0: p0 + pn])
                            # -- i,o,u pre-activations: one 6-bank psum --
                            pz = piou.tile([128, 6, TILE], fp32, tag="pz")
                            for g in range(3):          # i, o, u
                                for c in range(2):
                                    s = g * 2 + c
                                    m0 = g * 256 + c * 128
                                    for kc in range(3):
                                        nc.tensor.matmul(
                                            pz[:, s, :tn],
                                            wx_sb[:, kc, m0:m0 + 128],
                                            xt[:, kc, :tn],
                                            start=(kc == 0),
                                            stop=(kc == 2 and is_leaf))
                                    if not is_leaf:
                                        for kc in range(2):
                                            nc.tensor.matmul(
                                                pz[:, s, :tn],
                                                wh_sb[:, kc, m0:m0 + 128],
                                                hsum_cur[:, kc, t0:t0 + tn],
                                                start=False, stop=(kc == 1))
                            iot = gpool.tile([128, 4, TILE], bf16, tag="iot")
                            us = gpool.tile([128, 2, TILE], fp32, tag="us")
                            nc.scalar.activation(iot[:, :, :tn],
                                                 pz[:, 0:4, :tn], SIG)
                            nc.scalar.activation(us[:, :, :tn],
                                                 pz[:, 4:6, :tn], SIG,
                                                 scale=2.0)
                            ut = gpool.tile([128, 2, TILE], bf16, tag="ut")
                            nc.vector.tensor_scalar(ut[:, :, :tn],
                                                    us[:, :, :tn],
                                                    2.0, -1.0, op0=MUL,
                                                    op1=ADD)
                            # -- cell state / hidden --
                            ct = gpool.tile([128, 2, TILE], bf16, tag="ct")
                            nc.vector.tensor_mul(ct[:, :, :tn],
                                                 iot[:, 0:2, :tn],
                                                 ut[:, :, :tn])
                            if not is_leaf:
                                nc.vector.tensor_add(
                                    ct[:, :, :tn], ct[:, :, :tn],
                                    fcsum_cur[:, :, t0:t0 + tn])
                            cs = gpool.tile([128, 2, TILE], fp32, tag="cs")
                            nc.scalar.activation(cs[:, :, :tn], ct[:, :, :tn],
                                                 SIG, scale=2.0)
                            tct = gpool.tile([128, 2, TILE], bf16, tag="tct")
                            nc.vector.tensor_scalar(tct[:, :, :tn],
                                                    cs[:, :, :tn],
                                                    2.0, -1.0, op0=MUL,
                                                    op1=ADD)
                            ht = hpool.tile([128, 2, TILE], bf16, tag="ht")
                            nc.vector.tensor_mul(ht[:, :, :tn],
                                                 iot[:, 2:4, :tn],
                                                 tct[:, :, :tn])
                            # -- forget gates + child-pair sums --
                            if not is_root:
                                hv = ht[:, :, :tn].rearrange(
                                    "p c (n two) -> p c n two", two=2)
                                cv2 = ct[:, :, :tn].rearrange(
                                    "p c (n two) -> p c two n", two=2)
                                pf = pfl.tile([128, 2, TILE], fp32, tag="pfl")
                                for c in range(2):
                                    m0 = 768 + c * 128
                                    for kc in range(3):
                                        for par in range(2):
                                            # kc0/par0 start=True clears the
                                            # whole bank; later first-writes
                                            # to cleared cols overwrite.
                                            nc.tensor.matmul(
                                                pf[:, c,
                                                   par * pn:(par + 1) * pn],
                                                wx_sb[:, kc, m0:m0 + 128],
                                                xp[:, kc, :pn],
                                                start=(kc == 0 and par == 0),
                                                stop=False)
                                    for kc in range(2):
                                        for par in range(2):
                                            nc.tensor.matmul(
                                                pf[:, c,
                                                   par * pn:(par + 1) * pn],
                                                wh_sb[:, kc, m0:m0 + 128],
                                                hv[:, kc, :, par],
                                                start=False,
                                                stop=(kc == 1 and par == 1))
                                ft = gpool.tile([128, 2, TILE], bf16,
                                                tag="ft")
                                nc.scalar.activation(ft[:, :, :tn],
                                                     pf[:, :, :tn], SIG)
                                fct = gpool.tile([128, 2, TILE], bf16,
                                                 tag="fct")
                                fv = fct[:, :, :tn].rearrange(
                                    "p c (two n) -> p c two n", two=2)
                                nc.vector.tensor_mul(fv, ft[:, :, :tn]
                                                     .rearrange(
                                                         "p c (two n) -> "
                                                         "p c two n", two=2),
                                                     cv2)
                                q0 = t0 // 2
                                nc.vector.tensor_add(
                                    hsum_next[:, :, q0:q0 + pn],
                                    hv[:, :, :, 0], hv[:, :, :, 1])
                                nc.vector.tensor_add(
                                    fcsum_next[:, :, q0:q0 + pn],
                                    fct[:, :, 0:pn], fct[:, :, pn:2 * pn])
                            else:
                                nc.vector.tensor_copy(
                                    outhc_sb[:, 0:2 * nroot].rearrange(
                                        "p (c n) -> p c n", c=2),
                                    ht[:, :, :nroot])
                                nc.vector.tensor_copy(
                                    outhc_sb[:, 2 * nroot:4 * nroot]
                                    .rearrange("p (c n) -> p c n", c=2),
                                    ct[:, :, :nroot])
                            # -- logits: Wout-stationary, [5, tn] psum --
                            pl = pfl.tile([5, TILE], fp32, tag="pfl")
                            nc.tensor.matmul(pl[:, :tn], wo_sb[:, 0:5],
                                             ht[:, 0, :tn],
                                             start=True, stop=False)
                            nc.tensor.matmul(pl[:, :tn], wo_sb[:, 5:10],
                                             ht[:, 1, :tn],
                                             start=False, stop=True)
                            nc.vector.tensor_scalar(
                                stage_sb[0:5, off + t0:off + t0 + tn],
                                pl[:, :tn], bout5_sb[:], None, op0=ADD)
                        if not is_root:
                            hsum_cur, fcsum_cur = hsum_next, fcsum_next
                # --- epilogue: per-node logsumexp into stage row 5 ---
                ez = stpool.tile([5, nloc], bf16, tag="ez")
                nc.scalar.activation(ez[:], stage_sb[0:5, :], EXP)
                with tc.tile_pool(name="peps", bufs=2, space="PSUM") as peps:
                    nchunk = (nloc + EPSC - 1) // EPSC
                    for ch in range(nchunk):
                        c0 = ch * EPSC
                        cn = min(EPSC, nloc - c0)
                        pe = peps.tile([1, EPSC], fp32, tag="pe")
                        for q in range((cn + TILE - 1) // TILE):
                            q0 = q * TILE
                            qn = min(TILE, cn - q0)
                            nc.tensor.matmul(pe[:, q0:q0 + qn], ones5_sb[:],
                                             ez[:, c0 + q0:c0 + q0 + qn],
                                             start=True, stop=True)
                        nc.scalar.activation(lse_sb[:, c0:c0 + cn],
                                             pe[:, :cn], LN)
                nc.sync.dma_start(out5.ap()[0:5, :], stage_sb[0:5, :])
                nc.sync.dma_start(out5.ap()[5:6, :], lse_sb[:])
                nc.sync.dma_start(outhc.ap(), outhc_sb[:])

            if repeats == 1:
                body()
            else:
                engs = (mybir.EngineType.PE, mybir.EngineType.Activation,
                        mybir.EngineType.DVE, mybir.EngineType.SP,
                        mybir.EngineType.Pool)
                with tc.For_i(0, repeats, 1, hint_engines=engs):
                    body()
    nc.compile()
    _NC_CACHE[key] = nc
    return nc


# ---------------------------------------------------------------------------
# Host-side packing
# ---------------------------------------------------------------------------
def _core_node_index(core_depth=CORE_DEPTH, ncores=NCORES):
    """Global heap indices owned by core k, level-major (leaf level first)."""
    per_core = []
    top = DEPTH - core_depth
    for k in range(ncores):
        parts = []
        for d in range(DEPTH - 1, top - 1, -1):
            s = (1 << d) - 1
            m = 1 << (d - 3)           # per-core width at depth d
            parts.append(np.arange(s + k * m, s + (k + 1) * m))
        per_core.append(np.concatenate(parts))
    return per_core


def _pack_weights(inp):
    f32 = np.float32
    Wx = np.vstack([inp["W_ix"], inp["W_ox"], inp["W_ux"], inp["W_fx"]])
    Wh = np.vstack([inp["W_ih"], inp["W_oh"], inp["W_uh"], inp["W_fh"]])
    b_i = inp["b_ix"] + inp["b_ih"]
    b_o = inp["b_ox"] + inp["b_oh"]
    b_u = inp["b_ux"] + inp["b_uh"]
    b_f = inp["b_fx"] + inp["b_fh"]
    WxT = np.zeros((384, 1024), f32)
    WxT[:E] = Wx.T
    WxT[E] = np.concatenate([b_i, b_o, b_u, b_f])   # ones-row bias
    WhT = np.ascontiguousarray(Wh.T)  # [256, 1024]
    wxp = WxT.reshape(3, 128, 1024).transpose(1, 0, 2).reshape(128, 3 * 1024)
    whp = WhT.reshape(2, 128, 1024).transpose(1, 0, 2).reshape(128, 2 * 1024)
    WoT = np.ascontiguousarray(inp["W_out"].T)  # [256, 5]
    wop = WoT.reshape(2, 128, 5).transpose(1, 0, 2).reshape(128, 10)
    return {
        "wx": wxp.astype(BF16), "wh": whp.astype(BF16),
        "wo": wop.astype(BF16),
        "bout5": np.ascontiguousarray(
            inp["b_out"].reshape(5, 1).astype(f32)),
    }


def _pack_x(x, idx, nloc):
    xTp = np.zeros((384, nloc), BF16)
    xTp[:E] = x[idx].T.astype(BF16)
    xTp[E] = 1.0                                    # ones-row for bias
    return np.ascontiguousarray(
        xTp.reshape(3, 128, nloc).transpose(1, 0, 2).reshape(128, 3 * nloc))


def _host_top(inp, h_roots, c_roots, core_depth=CORE_DEPTH):
    """fp32 LSTM for the tree top (global levels above the subtree roots) +
    the subtree-root forget gates. Returns log-softmax rows for those nodes."""
    top = DEPTH - core_depth           # depth of subtree roots
    ntop = (1 << top) - 1              # nodes strictly above the roots
    x = np.asarray(inp["x"], np.float32)

    def sig(z):
        return 1.0 / (1.0 + np.exp(-z))

    h_sum = np.zeros((ntop, H), np.float32)
    fc_sum = np.zeros((ntop, H), np.float32)
    h_all = np.zeros((ntop, H), np.float32)
    for k in range(h_roots.shape[0]):
        g = ntop + k                  # global index of subtree root k
        p = (g - 1) // 2
        hk, ck = h_roots[k], c_roots[k]
        xf = x[p] @ inp["W_fx"].T + inp["b_fx"]
        f = sig(xf + hk @ inp["W_fh"].T + inp["b_fh"])
        h_sum[p] += hk
        fc_sum[p] += f * ck
    for d in range(top - 1, -1, -1):
        s, e = (1 << d) - 1, (1 << (d + 1)) - 1
        hs = h_sum[s:e]
        i = sig(x[s:e] @ inp["W_ix"].T + inp["b_ix"]
                + hs @ inp["W_ih"].T + inp["b_ih"])
        o = sig(x[s:e] @ inp["W_ox"].T + inp["b_ox"]
                + hs @ inp["W_oh"].T + inp["b_oh"])
        u = np.tanh(x[s:e] @ inp["W_ux"].T + inp["b_ux"]
                    + hs @ inp["W_uh"].T + inp["b_uh"])
        c = i * u + fc_sum[s:e]
        h = o * np.tanh(c)
        h_all[s:e] = h
        if d > 0:
            p = (np.arange(s, e) - 1) // 2
            xf = x[p] @ inp["W_fx"].T + inp["b_fx"]
            f = sig(xf + h @ inp["W_fh"].T + inp["b_fh"])
            np.add.at(h_sum, p, h)
            np.add.at(fc_sum, p, f * c)
    logits = h_all @ inp["W_out"].T + inp["b_out"]
    m = logits.max(-1, keepdims=True)
    lse = m + np.log(np.exp(logits - m).sum(-1, keepdims=True))
    return logits - lse


# ---------------------------------------------------------------------------
# Entry point
# ---------------------------------------------------------------------------
def kernel(**inputs):
    from concourse.bass_utils import run_bass_kernel_spmd

    inp = {k: np.asarray(v) for k, v in inputs.items()}
    sizes = _level_sizes(CORE_DEPTH)
    offs, nloc = _level_offsets(sizes)
    nroot = sizes[-1]
    nc = build_nc(CORE_DEPTH)

    w = _pack_weights(inp)
    idxs = _core_node_index()
    in_maps = []
    for k in range(NCORES):
        m = dict(w)
        m["xk"] = _pack_x(inp["x"], idxs[k], nloc)
        in_maps.append(m)
    res = run_bass_kernel_spmd(nc, in_maps, list(range(NCORES)))

    N = inp["x"].shape[0]
    out = np.zeros((N, 5), np.float32)
    h_roots = np.zeros((NCORES * nroot, H), np.float32)
    c_roots = np.zeros((NCORES * nroot, H), np.float32)
    for k in range(NCORES):
        r = res.results[k]
        o5 = np.asarray(r["out5"], np.float32)     # [6, nloc]
        out[idxs[k]] = (o5[0:5, :] - o5[5:6, :]).T
        hc = np.asarray(r["outhc"], np.float32)    # [128, 4*nroot]
        h = hc[:, 0:2 * nroot].reshape(128, 2, nroot)
        c = hc[:, 2 * nroot:4 * nroot].reshape(128, 2, nroot)
        for j in range(nroot):
            h_roots[k * nroot + j] = h[:, :, j].T.reshape(-1)
            c_roots[k * nroot + j] = c[:, :, j].T.reshape(-1)
    top = DEPTH - CORE_DEPTH
    out[: (1 << top) - 1] = _host_top(inp, h_roots, c_roots)
    return out
